# revision 8
# baseline (speedup 1.0000x reference)
"""Trainium2 Bass kernel for nn_DPConv_39771397161537 (self-contained).

Math (verified vs reference to 1e-7):
  xc = 1x1-conv(x) (64->16 ch) + bias
  xr[h,w] = xc[max(h-1,0), max(w-1,0)]  for h,w in [0,256)   (edge-pad + crop)
  for k in {4,8,12}: windows of xr at stride 4 (oh = 64/63/62), each k x k
  window mapped to 8x8 via a separable 2-tap filter B_k:
    k=4 : bilinear taps  t4  = [0,0,0,1,1,2,2,3], weights (.75,.25)/(.25,.75),
          edges p=0/7 single-tap 1.0
    k=8 : identity
    k=12: adaptive-avg taps t12 = [0,1,3,4,6,7,9,10], weights (.5,.5)
  out[n, l, c, p, q], windows concatenated: k4 rows [0,4096), k8 [4096,8065),
  k12 [8065,11909).

Per-core pipeline (batch-parallel, core n handles x[n]):
  conv (PE, x-chunk as stationary) -> xr in SBUF as [w-part, (c, h~)] with
  h~ = h+1, col 0 duplicated (absorbs the clamp);
  per scale: v-mix (DVE) -> Vm[w, (i,c,p)] ; PE transpose -> PSUM -> ACT evac
  -> VmT[(c,p), (i, w~)] (w~ = w+1, col 0 dup, absorbs w-clamp);
  h-mix (DVE) -> Hm[(c,p), (i,j,q)] ; PE matmul with per-q diagonal scale
  (folds the 2-tap normalizations) -> PSUM[(i2,j), (c,p,q)] ; ACT evac ->
  contiguous DMA store of 2*ow windows x 4KB.
"""

import dataclasses

import numpy as np

import concourse.bass as bass
import concourse.mybir as mybir
import concourse.tile as tile
from concourse import bacc, bass_utils

F32 = mybir.dt.float32

N_CORES = 8
CIN, COUT, H, W = 64, 16, 256, 256
OHS = {4: 64, 8: 63, 12: 62}
L_OFF = {4: 0, 8: 4096, 12: 8065}
L_TOT = 11909
BI = 8  # i-block size

# per-output-index 2-tap op specs: (a, b, sc) -> out = X[a]*sc + X[b];
# a=None -> out = X[b] (copy). Offsets are in window-tap space (h~ = 4i + off).
VOPS = {
    4: [(None, 0, 0.0), (1, 0, 1 / 3), (0, 1, 1 / 3), (2, 1, 1 / 3),
        (1, 2, 1 / 3), (3, 2, 1 / 3), (2, 3, 1 / 3), (None, 3, 0.0)],
    8: [(None, p, 0.0) for p in range(8)],
    12: [(t + 1, t, 1.0) for t in (0, 1, 3, 4, 6, 7, 9, 10)],
}

MULT = mybir.AluOpType.mult
ADD = mybir.AluOpType.add


def _sub(ap: bass.AP, off: int, dims) -> bass.AP:
    """Manual free-dim AP: keep partition dim of `ap`, replace free dims."""
    return dataclasses.replace(
        ap, offset=ap.offset + off, ap=[list(ap.ap[0])] + [list(d) for d in dims]
    )


def _mk(ap: bass.AP, off: int, dims) -> bass.AP:
    """Fully manual AP (DRAM side): replace all dims."""
    return dataclasses.replace(
        ap, offset=ap.offset + off, ap=[list(d) for d in dims]
    )


def make_consts(w_reduce: np.ndarray, b_reduce: np.ndarray) -> dict[str, np.ndarray]:
    wt = np.ascontiguousarray(w_reduce.T.astype(np.float32))  # [64, 16]
    consts = {
        "wred": np.concatenate([wt, wt], axis=0),  # [128, 16] rows hh*64+cin
        # bias replicated on all partitions, free layout (h8, c)
        "brep": np.broadcast_to(
            np.tile(b_reduce.astype(np.float32), 16), (128, 256)
        ).copy(),
        "ident": np.eye(128, dtype=np.float32),
    }
    identE = np.zeros((128, 129), np.float32)
    identE[0, 0] = 1.0
    identE[:, 1:] = np.eye(128, dtype=np.float32)
    consts["identE"] = identE
    s4 = np.array([1, 0.75, 0.75, 0.75, 0.75, 0.75, 0.75, 1], np.float32)
    dq4 = np.zeros((8, 128, 128), np.float32)
    for q in range(8):
        d = np.zeros(128, np.float32)
        for c in range(16):
            for p in range(8):
                d[c * 8 + p] = s4[p] * s4[q]
        dq4[q] = np.diag(d)
    consts["dq4"] = dq4
    consts["dq12"] = np.eye(128, dtype=np.float32) * 0.25
    return consts


def build_program():
    nc = bacc.Bacc(
        "TRN2", target_bir_lowering=False, debug=False, num_devices=N_CORES
    )
    x = nc.dram_tensor("x", (CIN, H, W), F32, kind="ExternalInput").ap()
    wred_d = nc.dram_tensor("wred", (128, 16), F32, kind="ExternalInput").ap()
    brep_d = nc.dram_tensor("brep", (128, 256), F32, kind="ExternalInput").ap()
    ident_d = nc.dram_tensor("ident", (128, 128), F32, kind="ExternalInput").ap()
    identE_d = nc.dram_tensor("identE", (128, 129), F32, kind="ExternalInput").ap()
    dq4_d = nc.dram_tensor("dq4", (8, 128, 128), F32, kind="ExternalInput").ap()
    dq12_d = nc.dram_tensor("dq12", (128, 128), F32, kind="ExternalInput").ap()
    out_d = nc.dram_tensor("out", (L_TOT, 1024), F32, kind="ExternalOutput").ap()

    with tile.TileContext(nc) as tc:
        with tc.tile_pool(name="const", bufs=1) as cpool, tc.tile_pool(
            name="perm", bufs=1
        ) as perm:
            wred = cpool.tile([128, 16], F32)
            nc.sync.dma_start(out=wred, in_=wred_d)
            brep = cpool.tile([128, 16, 16], F32)
            nc.sync.dma_start(out=brep, in_=brep_d.rearrange("p (a b) -> p a b", b=16))
            ident = cpool.tile([128, 128], F32)
            nc.sync.dma_start(out=ident, in_=ident_d)
            identE = cpool.tile([128, 129], F32)
            nc.sync.dma_start(out=identE, in_=identE_d)
            dq4 = cpool.tile([128, 8, 128], F32)
            nc.sync.dma_start(out=dq4, in_=dq4_d.rearrange("q r c -> r q c"))
            dq12 = cpool.tile([128, 128], F32)
            nc.sync.dma_start(out=dq12, in_=dq12_d)

            # xr tiles: [w-part (2 chunks), c, h~(260)]; h~ = h+1, col0 dup
            xr = [perm.tile([128, 16, 260], F32, name=f"xr{wc}") for wc in (0, 1)]

            # ---------------- conv ----------------
            with tc.tile_pool(name="xin", bufs=3) as xpool, tc.tile_pool(
                name="cpsum", bufs=2, space="PSUM"
            ) as cps_pool:
                for g in range(8):
                    xt = xpool.tile([128, 16, 256], F32, tag="xt")
                    # src order (hh, c, e, w): x[c, g*32 + hh*16 + e, w]
                    xsrc = _mk(
                        x, g * 32 * 256,
                        [[16 * 256, 2], [256 * 256, 64], [256, 16], [1, 256]],
                    )
                    nc.sync.dma_start(out=xt, in_=xsrc)
                    for hh in range(2):
                        for wc in range(2):
                            # bank-aligned psum slot: (h8, 32) stride, cols 0:16 used
                            ps = cps_pool.tile([128, 16, 32], F32, tag="cps")
                            for h8 in range(16):
                                nc.tensor.matmul(
                                    ps[:, h8, 0:16],
                                    xt[hh * 64 : hh * 64 + 64, h8, wc * 128 : wc * 128 + 128],
                                    wred[hh * 64 : hh * 64 + 64, :],
                                    start=True,
                                    stop=True,
                                    skip_group_check=True,
                                )
                            # evac + bias: xr[c, h~] at h~ = 1 + g*32 + hh*16 + h8
                            h0 = 1 + g * 32 + hh * 16
                            dst = _sub(xr[wc], h0, [[1, 16], [260, 16]])  # (h8, c)
                            nc.vector.scalar_tensor_tensor(
                                dst, ps[:, :, 0:16], 1.0, brep, MULT, ADD
                            )
                            if g == 0 and hh == 0:
                                # h~ = 0 dup of h~ = 1 (clamp row)
                                nc.scalar.copy(
                                    out=xr[wc][:, :, 0:1], in_=xr[wc][:, :, 1:2]
                                )

            # ---------------- scales ----------------
            with tc.tile_pool(name="sc", bufs=2) as sp, tc.tile_pool(
                name="so", bufs=3
            ) as sop, tc.tile_pool(name="t1ps", bufs=1, space="PSUM") as t1p, tc.tile_pool(
                name="t2ps", bufs=2, space="PSUM"
            ) as t2p:
                for K in (4, 8, 12):
                    OW = OHS[K]
                    nblk = (OW + BI - 1) // BI
                    for ib in range(nblk):
                        i0 = ib * BI
                        ni = min(BI, OW - i0)
                        # ---- stage A: v-mix -> Vm [128, wc, i, c, p]
                        # Vm free strides: wc 1024, i 128, c 8, p 1
                        Vm = sp.tile([128, 2, BI, 16, 8], F32, tag="vm", name="Vm")
                        for wc in (0, 1):
                            xrt = xr[wc]
                            vbase = wc * 1024
                            for p in range(8):
                                a, b, sc = VOPS[K][p]
                                dst = _sub(Vm, vbase + p, [[128, ni], [8, 16]])
                                if a is None:
                                    eng = nc.gpsimd if K == 8 else nc.vector
                                    eng.tensor_copy(
                                        dst,
                                        _sub(xrt, 4 * i0 + b, [[4, ni], [260, 16]]),
                                    )
                                else:
                                    nc.vector.scalar_tensor_tensor(
                                        dst,
                                        _sub(xrt, 4 * i0 + a, [[4, ni], [260, 16]]),
                                        sc,
                                        _sub(xrt, 4 * i0 + b, [[4, ni], [260, 16]]),
                                        MULT,
                                        ADD,
                                    )
                        # ---- stage B/C: transpose + evac -> VmT [(c,p), i, w~]
                        VmT = sp.tile([128, BI, 260], F32, tag="vmt", name="VmT")
                        for t0 in range(0, ni, 2):
                            nt = min(2, ni - t0)
                            t1 = t1p.tile([128, 2, 512], F32, tag="t1", name="T1ps")
                            for dt in range(nt):
                                i = t0 + dt
                                for wc in (0, 1):
                                    lhsT = _sub(
                                        Vm, wc * (BI * 128) + i * 128, [[1, 128]]
                                    )
                                    nc.tensor.matmul(
                                        _sub(
                                            t1,
                                            dt * 512 + (0 if wc == 0 else 129),
                                            [[1, 129 if wc == 0 else 128]],
                                        ),
                                        lhsT,
                                        identE if wc == 0 else ident,
                                        start=True,
                                        stop=True,
                                        is_transpose=(wc == 1),
                                        skip_group_check=True,
                                    )
                            nc.scalar.copy(
                                out=_sub(VmT, t0 * 260, [[260, nt], [1, 257]]),
                                in_=_sub(t1, 0, [[512, nt], [1, 257]]),
                            )
                        # ---- stage D: h-mix -> Hm [(c,p), i, j, q]
                        # Hm free strides: i OW*8, j 8, q 1
                        Hm = sp.tile([128, BI, OW, 8], F32, tag="hm", name="Hm")
                        for q in range(8):
                            a, b, sc = VOPS[K][q]
                            dst = _sub(Hm, q, [[OW * 8, ni], [8, OW]])
                            if a is None:
                                eng = nc.gpsimd if K == 8 else nc.vector
                                eng.tensor_copy(
                                    dst, _sub(VmT, b, [[260, ni], [4, OW]])
                                )
                            else:
                                nc.vector.scalar_tensor_tensor(
                                    dst,
                                    _sub(VmT, a, [[260, ni], [4, OW]]),
                                    sc,
                                    _sub(VmT, b, [[260, ni], [4, OW]]),
                                    MULT,
                                    ADD,
                                )
                        # ---- stage E/F: per-(i-pair, q) matmul + evac + store
                        for t0 in range(0, ni, 2):
                            nt = min(2, ni - t0)
                            M = nt * OW
                            t2 = t2p.tile([128, 1030], F32, tag="t2", name="T2ps")
                            for q in range(8):
                                if K == 4:
                                    rhs = dq4[:, q]
                                elif K == 8:
                                    rhs = ident
                                else:
                                    rhs = dq12
                                lhsT = _sub(Hm, t0 * OW * 8 + q, [[8, M]])
                                for half in range(2):
                                    # psum col = c*64 + p*8 + q (final layout)
                                    nc.tensor.matmul(
                                        _sub(t2[0:M], half * 512 + q,
                                             [[64, 8], [8, 8]]),
                                        lhsT,
                                        rhs[:, half * 64 : half * 64 + 64],
                                        start=True,
                                        stop=True,
                                        skip_group_check=True,
                                    )
                            So = sop.tile([128, 1024], F32, tag="so", name="So")
                            nc.scalar.copy(out=So[0:M, :], in_=t2[0:M, 0:1024])
                            l0 = L_OFF[K] + (i0 + t0) * OW
                            nc.sync.dma_start(
                                out=out_d[l0 : l0 + M, :], in_=So[0:M, :]
                            )

    nc.compile()
    return nc


_NC = None


def _get_nc():
    global _NC
    if _NC is None:
        _NC = build_program()
    return _NC


def kernel(x: np.ndarray, w_reduce: np.ndarray, b_reduce: np.ndarray) -> np.ndarray:
    assert x.shape == (8, CIN, H, W), x.shape
    nc = _get_nc()
    consts = make_consts(np.asarray(w_reduce), np.asarray(b_reduce))
    in_maps = []
    for n in range(N_CORES):
        m = dict(consts)
        m["x"] = np.ascontiguousarray(np.asarray(x)[n], dtype=np.float32)
        in_maps.append(m)
    res = bass_utils.run_bass_kernel_spmd(nc, in_maps, core_ids=list(range(N_CORES)))
    out = np.stack([r["out"].reshape(L_TOT, COUT, 8, 8) for r in res.results])
    return out.astype(np.float32)
